# revision 8
# baseline (speedup 1.0000x reference)
"""Trainium2 Bass kernel for causal multi-head attention + output projection.

Problem: B=2, S=2048, D=1024, H=16 heads of HD=64; fp32; causal softmax
scaled by D**-0.5; output projection with bias.

Sharding: 2 heads per core (tensor parallel on heads) for QKV + attention.
Output rows are interleave-sharded at 64-column granularity (q-block g of 64
columns is owned by core g%8), so after EVERY 512-wide attention chunk the
cores exchange one small AllToAll ([8,128,64] bf16) instead of one big fp32
AllToAll per (batch, head-pair) at the end.  The producing core normalizes
(softmax denominator) and casts to bf16 BEFORE the exchange, so the
receiving core only runs output-projection matmuls.  This removes the
serial denominator/normalize chain from the kernel tail and overlaps all
communication with attention compute.

Math notes:
 - All attention tensors are kept transposed ([feature, seq] layouts) so
   every matmul contracts on the partition dim with zero on-chip transposes
   (except V, which is produced as V^T and transposed via the PE).
 - softmax is computed without max-subtraction: logits are N(0, 1/16) by
   construction, so exp() is numerically safe; the denominator is
   accumulated by a column of ones appended to V (row 64 of the O^T PSUM
   accumulator).
 - float32r (TF32-like) matmuls run at bf16 rate with ~1e-4 relative error.
"""

import sys

sys.path.insert(0, "/opt/trn_rl_repo")

import numpy as np

import concourse.bacc as bacc
import concourse.mybir as mybir
import concourse.tile as tile
from concourse.bass_utils import run_bass_kernel_spmd

B, D, H, HD = 2, 1024, 16, 64
NCORES = 8
SCALE = float(D) ** -0.5
F32 = mybir.dt.float32
F32R = mybir.dt.float32r
BF16 = mybir.dt.bfloat16
Exp = mybir.ActivationFunctionType.Exp


def build(S=2048, dump=False):
    KD = D // 128          # contraction tiles for the projections
    NT = S // 128          # key tiles
    SQ = 512               # query-chunk width
    NCH = S // SQ          # query chunks per (batch, head)
    SLOT = SQ // NCORES    # 64: per-core column slot inside a chunk
    NP = NCH // 2          # 128-row output "pairs" per batch per core
    YR = B * NP * 128      # output rows per core

    nc = bacc.Bacc("TRN2", target_bir_lowering=False, debug=False)
    xT = nc.dram_tensor("xT", [B, D, S], BF16, kind="ExternalInput")
    Wqkv = nc.dram_tensor("Wqkv", [128, 3, D // 128, 128], BF16, kind="ExternalInput")
    WpT = nc.dram_tensor("WpT", [128, D // 128, D], BF16, kind="ExternalInput")
    bp = nc.dram_tensor("bp", [1, D], BF16, kind="ExternalInput")
    mask = nc.dram_tensor("mask", [128, 128], BF16, kind="ExternalInput")
    idin = nc.dram_tensor("idin", [128, 128], BF16, kind="ExternalInput")
    # y rows (per core c): [b, p, j]: global q = SQ*(2p + j//64) + 64c + j%64
    y = nc.dram_tensor("y", [YR, D], F32, kind="ExternalOutput")

    with tile.TileContext(nc) as tc:
        ctx_pools = [
            tc.tile_pool(name="persist", bufs=1),
            tc.tile_pool(name="dram", bufs=1, space="DRAM"),
            tc.tile_pool(name="wq", bufs=1),
            tc.tile_pool(name="xp", bufs=2),
            tc.tile_pool(name="qk", bufs=2),
            tc.tile_pool(name="vp", bufs=2),
            tc.tile_pool(name="at", bufs=6),
            tc.tile_pool(name="fin", bufs=2),
            tc.tile_pool(name="yo", bufs=2),
            tc.tile_pool(name="ps_mix", bufs=2, space="PSUM"),
            tc.tile_pool(name="ps_sc", bufs=2, space="PSUM"),
            tc.tile_pool(name="ps_oT", bufs=2, space="PSUM"),
        ]
        import contextlib

        with contextlib.ExitStack() as stk:
            (
                persist, dram, wpool, xpool, qkpool, vppool, atpool,
                finpool, ypool, ps_mix, ps_sc, ps_oT,
            ) = [stk.enter_context(p) for p in ctx_pools]

            # ---- critical-path first: small constants, weights, batch-0 x ----
            ident = persist.tile([128, 128], BF16)
            nc.sync.dma_start(out=ident, in_=idin[:, :])
            mask_sb = persist.tile([128, 128], BF16)
            nc.sync.dma_start(out=mask_sb, in_=mask[:, :])
            wqkv_sb = wpool.tile([128, 3, KD, 128], BF16)
            nc.sync.dma_start(out=wqkv_sb, in_=Wqkv[:, :, :, :])
            # PE warm-up while input DMAs are in flight: ramps the clock gate
            wps = ps_sc.tile([128, 2, SQ], F32, tag="ps_sc", name="warmps")
            for _ in range(90):
                nc.tensor.matmul(wps[:, 0, 0:128], ident, ident, start=True, stop=True)

            def load_x(b):
                xs = [
                    xpool.tile([128, S], BF16, tag=f"x{t}", name=f"x_{b}_{t}")
                    for t in range(KD)
                ]
                for t in range(KD):
                    nc.sync.dma_start(
                        out=xs[t], in_=xT[b, 128 * t : 128 * (t + 1), :]
                    )
                return xs

            x_sb = {0: load_x(0)}

            ones_sb = persist.tile([1, 128], BF16)
            nc.vector.memset(ones_sb, 1.0)
            # all-ones row parked at partition 64 (to pair with PSUM row 64
            # of the O^T accumulator in the rcp-broadcast matmul)
            ones65 = persist.tile([65, 128], F32)
            nc.vector.memset(ones65[64:65, :], 1.0)

            # onrm[b]: [128 feat, KD sender, NCH chunk, SLOT] normalized bf16
            onrm = {
                b: persist.tile([128, KD, NCH, SLOT], BF16, name=f"onrm_{b}")
                for b in range(B)
            }
            a2a_in = {
                (b, n): dram.tile([NCORES, 2, 64, SLOT], BF16, name=f"a2a_in_{b}_{n}")
                for b in range(B)
                for n in range(NCH)
            }
            a2a_out = {
                (b, n): dram.tile([NCORES, 2, 64, SLOT], BF16, name=f"a2a_out_{b}_{n}")
                for b in range(B)
                for n in range(NCH)
            }
            qkvT = {}
            vp = {}
            ot_ps = {}     # (b, hs) -> live O^T psum tile of the current chunk
            fin_state = {}

            def emit_qkv_group(b, w, n, eng=None):
                if b not in qkvT:
                    qkvT[b] = qkpool.tile(
                        [128, 3, S], BF16, tag="qkvT", name=f"qkvT_{b}"
                    )
                pst = ps_mix.tile([128, SQ], F32, tag="mix", name=f"psqk_{b}_{w}_{n}")
                for t in range(KD):
                    nc.tensor.matmul(
                        pst,
                        wqkv_sb[:, w, t, :],
                        x_sb[b][t][:, SQ * n : SQ * (n + 1)],
                        start=(t == 0),
                        stop=(t == KD - 1),
                    )
                dst = qkvT[b][:, w, SQ * n : SQ * (n + 1)]
                if eng == "scalar":
                    nc.scalar.copy(dst, pst)
                else:
                    nc.vector.tensor_copy(dst, pst)

            def emit_v_unit(b, i):
                if b not in vp:
                    vp[b] = vppool.tile(
                        [128, NT, 2, 65], BF16, tag="vp", name=f"vp_{b}"
                    )
                    nc.vector.memset(vp[b][:, :, :, 64], 1.0)
                pst = ps_mix.tile([128, 128], BF16, tag="mix", name=f"psvt_{b}_{i}")
                nc.tensor.transpose(
                    pst, qkvT[b][:, 2, 128 * i : 128 * (i + 1)], ident[:, :]
                )
                for hs in range(2):
                    nc.vector.tensor_copy(
                        vp[b][:, i, hs, 0:64], pst[:, 64 * hs : 64 * hs + 64]
                    )

            def emit_attn_chunk(b, hs, n, fillers):
                qT = qkvT[b][64 * hs : 64 * hs + 64, 0, :]
                kT = qkvT[b][64 * hs : 64 * hs + 64, 1, :]
                ot = ps_oT.tile([65, SQ], F32, tag="ps_oT", name=f"ot_{b}_{hs}_{n}")
                ot_ps[(b, hs)] = ot
                jmax = 4 * n + 4
                for jp in range(0, jmax, 2):
                    sc = ps_sc.tile(
                        [128, 2, SQ], F32, tag="ps_sc", name=f"sc_{b}_{hs}_{n}_{jp}"
                    )
                    at = atpool.tile([128, 2, SQ], BF16, tag="at")
                    offs = []
                    for k in range(2):
                        j = jp + k
                        off = max(0, 128 * j - SQ * n)
                        offs.append(off)
                        nc.tensor.matmul(
                            sc[:, k, off:],
                            kT[:, 128 * j : 128 * (j + 1)],
                            qT[:, SQ * n + off : SQ * (n + 1)],
                            start=True,
                            stop=True,
                        )
                    # one exp over both halves (covers a dead zone between them)
                    o0 = offs[0]
                    nc.scalar.activation(
                        at[:, :, :].rearrange("p a s -> p (a s)")[:, o0:],
                        sc[:, :, :].rearrange("p a s -> p (a s)")[:, o0:],
                        Exp,
                        scale=SCALE,
                    )
                    for k in range(2):
                        j = jp + k
                        off = offs[k]
                        if j >= 4 * n:
                            nc.gpsimd.tensor_mul(
                                at[:, k, off : off + 128],
                                at[:, k, off : off + 128],
                                mask_sb,
                            )
                        nc.tensor.matmul(
                            ot[:, off:],
                            vp[b][:, j, hs, :],
                            at[:, k, off:],
                            start=(j == 0),
                            stop=(j == jmax - 1),
                        )
                    if fillers:
                        fillers[0] -= 2
                        if fillers[0] <= 0 and len(fillers) > 1:
                            fillers[0] = fillers.pop(1)
                            fillers.pop(1)()

            # ---- per-(b, n) finalize: normalize, pack bf16, exchange ----
            def emit_fin_rcp(b, n, hs):
                """Reciprocal of the denominator row (row 64 of the O^T psum).
                Emitted right after chunk (b, hs, n); non-PE ops only."""
                st = fin_state.setdefault((b, n), {})
                if "rcp" not in st:
                    st["rcp"] = finpool.tile(
                        [65, 2, SQ], F32R, tag="rcp", name=f"rcp_{b}_{n}"
                    )
                    st["bcs"] = finpool.tile(
                        [64, 2, SQ], F32, tag="bcs", name=f"bcs_{b}_{n}"
                    )
                    st["src"] = finpool.tile(
                        [64, 2, SQ], BF16, tag="src", name=f"src_{b}_{n}"
                    )
                with nc.allow_low_precision(reason="softmax denom recip"):
                    nc.vector.reciprocal(
                        st["rcp"][64:65, hs, :],
                        ot_ps[(b, hs)][64:65, :].bitcast(F32R),
                    )

            def emit_fin_norm(b, n, hs):
                """Broadcast 1/den over 64 partitions (PE outer product from
                partition 64 down to 0..63), then normalize+cast to bf16."""
                st = fin_state[(b, n)]
                bc = ps_mix.tile([64, SQ], F32, tag="mix", name=f"bc_{b}_{n}_{hs}")
                nc.tensor.matmul(
                    bc,
                    ones65[64:65, 0:64].bitcast(F32R),
                    st["rcp"][64:65, hs, :],
                    start=True,
                    stop=True,
                )
                nc.scalar.copy(st["bcs"][:, hs, :], bc)
                nc.vector.tensor_mul(
                    st["src"][:, hs, :],
                    ot_ps[(b, hs)][0:64, :],
                    st["bcs"][:, hs, :],
                )

            def emit_fin_xchg(b, n):
                """Stage the normalized chunk into DRAM slots + AllToAll +
                unpack into the onrm projection operand."""
                st = fin_state[(b, n)]
                for d in range(NCORES):
                    for hs in range(2):
                        nc.sync.dma_start(
                            out=a2a_in[(b, n)][d, hs, :, :],
                            in_=st["src"][:, hs, SLOT * d : SLOT * (d + 1)],
                        )
                nc.gpsimd.collective_compute(
                    "AllToAll",
                    mybir.AluOpType.bypass,
                    replica_groups=[list(range(NCORES))],
                    ins=[a2a_in[(b, n)][:, :, :, :].opt()],
                    outs=[a2a_out[(b, n)][:, :, :, :].opt()],
                )
                # unpack: sender t -> contraction block t of onrm
                nc.sync.dma_start(
                    out=onrm[b][:, :, n, :],
                    in_=a2a_out[(b, n)][:, :, :, :].rearrange("d h p c -> (h p) d c"),
                )

            # ---- output projection for one 128-row pair (chunks 2p, 2p+1) ----
            ydict = {}

            def emit_D_group(b, p, nn):
                acc = ps_mix.tile([128, 512], F32, tag="mix", name=f"acc_{b}_{p}_{nn}")
                for t in range(KD):
                    nc.tensor.matmul(
                        acc,
                        onrm[b][:, t, 2 * p : 2 * p + 2, :],
                        wpT_sb[:, t, 512 * nn : 512 * (nn + 1)],
                        start=(t == 0),
                        stop=False,
                    )
                nc.tensor.matmul(
                    acc, ones_sb, bp_sb[:, 512 * nn : 512 * (nn + 1)],
                    start=False, stop=True,
                )
                if (b, p) not in ydict:
                    ydict[(b, p)] = ypool.tile([128, D], F32, tag="y", name=f"y_{b}_{p}")
                ys = ydict[(b, p)]
                nc.vector.tensor_copy(ys[:, 512 * nn : 512 * (nn + 1)], acc)
                if nn == D // 512 - 1:
                    r0 = (b * NP + p) * 128
                    nc.sync.dma_start(out=y[r0 : r0 + 128, :], in_=ys)

            # ---- schedule ----
            # batch 0 projections (dense PE, warms HAM)
            for w in range(3):
                for n in range(NCH):
                    emit_qkv_group(0, w, n, eng="scalar")
            for i in range(NT):
                emit_v_unit(0, i)
            # batch-1 x load starts now (behind batch-0 x on the DMA queues)
            x_sb[1] = load_x(1)
            # deferred big loads for the output projection
            wpT_sb = persist.tile([128, KD, D], BF16)
            nc.sync.dma_start(out=wpT_sb, in_=WpT[:, :, :])
            bp_sb = persist.tile([1, D], BF16)
            nc.sync.dma_start(out=bp_sb, in_=bp[:, :])

            # ---- filler plan ----
            # linear chunk order; chunk ci=(b,n) gets woven-in units:
            #  - tail of the previous chunk (hs1 normalize + exchange)
            #  - its static unit list (b1 prep during b0, D groups later)
            chunks = [(b, n) for b in range(B) for n in range(NCH)]
            upc = {ci: [] for ci in chunks}

            b1_units = []
            for w in range(3):
                for n in range(NCH):
                    b1_units.append(lambda w=w, n=n: emit_qkv_group(1, w, n))
            for i in range(NT):
                b1_units.append(lambda i=i: emit_v_unit(1, i))
            # distribute b1 prep units over batch-0 chunks, weighted by size
            w0 = [4 * n + 4 for n in range(NCH)]
            tot0 = sum(w0)
            pos = 0
            for n in range(NCH):
                take = max(1, round(len(b1_units) * w0[n] / tot0))
                if n == NCH - 1:
                    take = len(b1_units) - pos
                upc[(0, n)].extend(b1_units[pos : pos + take])
                pos += take

            # D(b, p) needs the exchange of chunks (b,2p) and (b,2p+1); the
            # exchange of (b,m) is emitted at the START of chunk index
            # idx(b,m)+1, so D units are safe from chunk index idx(b,2p+1)+2.
            tail_d = []
            for b in range(B):
                for p in range(NP):
                    idx = b * NCH + (2 * p + 1) + 2
                    units = [
                        lambda b=b, p=p, nn=nn: emit_D_group(b, p, nn)
                        for nn in range(D // 512)
                    ]
                    if idx < len(chunks):
                        upc[chunks[idx]].extend(units)
                    else:
                        tail_d.extend(units)

            gf = [4]  # global filler list: [countdown, unit, countdown, ...]

            def gf_add(units, total_j):
                pace = max(2, total_j // max(len(units), 1))
                for u in units:
                    gf.extend([pace, u])

            import os
            safe = os.environ.get("K_SAFE", "0")
            prev = None
            for (b, n) in chunks:
                if (b, n) == (1, 0):
                    # all batch-1 prep (qkv/v) must be emitted before batch-1
                    # attention reads it: force-drain leftover fillers
                    while len(gf) > 1:
                        gf.pop(1)
                        gf.pop(1)()
                    gf[:] = [4]
                units = []
                if prev is not None:
                    pb, pn = prev

                    def prev_tail(pb=pb, pn=pn):
                        emit_fin_norm(pb, pn, 1)
                        emit_fin_xchg(pb, pn)

                    if safe in ("1", "3"):
                        prev_tail()
                    else:
                        units.append(prev_tail)
                units.extend(upc[(b, n)])
                # hs0-normalize of THIS chunk is woven into hs1 below
                total_j = 2 * (4 * n + 4)
                gf_add(units, total_j)
                emit_attn_chunk(b, 0, n, gf)
                emit_fin_rcp(b, n, 0)
                if safe in ("1", "2"):
                    emit_fin_norm(b, n, 0)
                    emit_attn_chunk(b, 1, n, gf)
                else:
                    # weave hs0's normalize early into the hs1 chunk
                    gf.insert(1, lambda b=b, n=n: emit_fin_norm(b, n, 0))
                    gf.insert(1, 2)
                    emit_attn_chunk(b, 1, n, gf)
                emit_fin_rcp(b, n, 1)
                prev = (b, n)

            # drain leftover fillers
            while len(gf) > 1:
                gf.pop(1)
                gf.pop(1)()
            # tail: last chunk's normalize + exchange + final D groups
            emit_fin_norm(1, NCH - 1, 1)
            emit_fin_xchg(1, NCH - 1)
            for u in tail_d:
                u()

    nc.compile()
    return nc


_built = {}


def get_nc(S=2048):
    if S not in _built:
        _built[S] = build(S)
    return _built[S]


def prep_inputs(x, Wq, Wk, Wv, Wp, bp):
    """Host-side shard prep. Returns per-core input maps."""
    import ml_dtypes

    BF = ml_dtypes.bfloat16
    x = np.ascontiguousarray(np.asarray(x, dtype=np.float32))
    Wq, Wk, Wv = (np.asarray(w, dtype=np.float32) for w in (Wq, Wk, Wv))
    Wp = np.asarray(Wp, dtype=np.float32)
    bp = np.asarray(bp, dtype=np.float32)
    BFc = BF
    xT = np.ascontiguousarray(x.transpose(0, 2, 1)).astype(BFc)
    KD = D // 128
    # WpT pre-arranged for SBUF: [p, t, i] with row t*128+p of Wp.T
    WpT = np.ascontiguousarray(
        Wp.T.reshape(KD, 128, D).transpose(1, 0, 2)
    ).astype(BFc)
    mask = np.triu(np.ones((128, 128), dtype=np.float32)).astype(BFc)
    idin = np.eye(128, dtype=np.float32).astype(BFc)
    in_maps = []
    for c in range(NCORES):
        h0 = 2 * c
        wqkv = np.stack(
            [
                np.concatenate([Wq[h0], Wq[h0 + 1]], axis=1),
                np.concatenate([Wk[h0], Wk[h0 + 1]], axis=1),
                np.concatenate([Wv[h0], Wv[h0 + 1]], axis=1),
            ]
        )  # [3, D, 128]
        # pre-arrange: [p, w, t, m]
        wqkv = np.ascontiguousarray(
            wqkv.reshape(3, KD, 128, 128).transpose(2, 0, 1, 3)
        ).astype(BF)
        in_maps.append(
            {
                "xT": xT,
                "Wqkv": wqkv,
                "WpT": WpT,
                "bp": bp.reshape(1, D).astype(BF),
                "mask": mask,
                "idin": idin,
            }
        )
    return in_maps


# inputs identical across cores are passed replicated (shipped once, not 8x)
_REPLICATED = {"xT", "WpT", "bp", "mask", "idin"}

_runners = {}


def _get_runner(S):
    """Cached jitted SPMD callable for the built module."""
    if S in _runners:
        return _runners[S]
    import jax
    import concourse.mybir as _mybir
    from concourse import bass2jax
    from jax.experimental.shard_map import shard_map
    from jax.sharding import Mesh, PartitionSpec

    nc = get_nc(S)
    bass2jax.install_neuronx_cc_hook()

    in_names, out_names, out_avals = [], [], []
    partition_name = nc.partition_id_tensor.name if nc.partition_id_tensor else None
    for alloc in nc.m.functions[0].allocations:
        if not isinstance(alloc, _mybir.MemoryLocationSet):
            continue
        name = alloc.memorylocations[0].name
        if alloc.kind == "ExternalInput":
            if name != partition_name:
                in_names.append(name)
        elif alloc.kind == "ExternalOutput":
            out_names.append(name)
            out_avals.append(
                jax.core.ShapedArray(tuple(alloc.tensor_shape), _mybir.dt.np(alloc.dtype))
            )
    n_params = len(in_names)
    all_in_names = list(in_names) + list(out_names)
    if partition_name is not None:
        all_in_names.append(partition_name)

    def _body(*args):
        operands = list(args)
        if partition_name is not None:
            operands.append(bass2jax.partition_id_tensor())
        outs = bass2jax._bass_exec_p.bind(
            *operands,
            out_avals=tuple(out_avals),
            in_names=tuple(all_in_names),
            out_names=tuple(out_names),
            lowering_input_output_aliases=(),
            sim_require_finite=True,
            sim_require_nnan=True,
            nc=nc,
        )
        return tuple(outs)

    devices = jax.devices()[:NCORES]
    mesh = Mesh(np.asarray(devices), ("core",))
    in_specs = tuple(
        PartitionSpec() if nm in _REPLICATED else PartitionSpec("core")
        for nm in in_names
    ) + (PartitionSpec("core"),) * len(out_names)
    out_specs = (PartitionSpec("core"),) * len(out_names)
    donate = tuple(range(n_params, n_params + len(out_names)))
    fn = jax.jit(
        shard_map(_body, mesh=mesh, in_specs=in_specs, out_specs=out_specs, check_rep=False),
        donate_argnums=donate,
        keep_unused=True,
    )
    r = (fn, in_names, out_names, out_avals, mesh)
    _runners[S] = r
    return r


class _Res:
    def __init__(self, results):
        self.results = results
        self.exec_time_ns = None


def run(x, Wq, Wk, Wv, Wp, bp, timings=None):
    import time as _time

    S = x.shape[1]
    t0 = _time.perf_counter()
    fn, in_names, out_names, out_avals, mesh = _get_runner(S)
    t1 = _time.perf_counter()
    in_maps = prep_inputs(x, Wq, Wk, Wv, Wp, bp)
    t2 = _time.perf_counter()
    args = []
    for nm in in_names:
        if nm in _REPLICATED:
            args.append(in_maps[0][nm])
        else:
            args.append(np.concatenate([in_maps[c][nm] for c in range(NCORES)], axis=0))
    zero_outs = [
        np.zeros((NCORES * av.shape[0], *av.shape[1:]), av.dtype) for av in out_avals
    ]
    t3 = _time.perf_counter()
    out_arrs = fn(*args, *zero_outs)
    out_np = [np.asarray(o) for o in out_arrs]
    t4 = _time.perf_counter()
    results = [
        {
            nm: out_np[i].reshape(NCORES, *out_avals[i].shape)[c]
            for i, nm in enumerate(out_names)
        }
        for c in range(NCORES)
    ]
    if timings is not None:
        timings.update(
            runner=t1 - t0, prep=t2 - t1, concat=t3 - t2, exec=t4 - t3
        )
    return _assemble_y([results[c]["y"] for c in range(NCORES)], S), _Res(results)


def _assemble_y(per_core, S):
    """per-core y rows [b, p, j]: global q = SQ*(2p + j//64) + 64c + j%64."""
    SQ = 512
    NCH = S // SQ
    NP = NCH // 2
    out = np.empty((B, S, D), dtype=per_core[0].dtype)
    for c in range(NCORES):
        yc = per_core[c].reshape(B, NP, 128, D)
        for b in range(B):
            for p in range(NP):
                for half in range(2):
                    q0 = SQ * (2 * p + half) + 64 * c
                    out[b, q0 : q0 + 64, :] = yc[b, p, 64 * half : 64 * half + 64]
    return out


def kernel(x, Wq, Wk, Wv, Wp, bp):
    out, _ = run(x, Wq, Wk, Wv, Wp, bp)
    return out


# ---------------------------------------------------------------------------
# NTFF profiling support (test harness only; not needed for kernel()).
# ---------------------------------------------------------------------------

def _ntff_hook():
    import contextlib
    import ctypes

    lib = ctypes.CDLL("/opt/axon/libaxon_pjrt.so")
    lib.axon_start_nrt_profile.argtypes = [
        ctypes.POINTER(ctypes.c_int64),
        ctypes.c_size_t,
    ]
    lib.axon_start_nrt_profile.restype = ctypes.c_int64
    lib.axon_stop_nrt_profile.argtypes = [ctypes.c_char_p]
    lib.axon_stop_nrt_profile.restype = ctypes.c_int64

    @contextlib.contextmanager
    def _hook(output_dir, device_ids):
        import jax

        jax.devices()
        if device_ids:
            ids = (ctypes.c_int64 * len(device_ids))(*device_ids)
            rc = lib.axon_start_nrt_profile(ids, len(device_ids))
        else:
            rc = lib.axon_start_nrt_profile(None, 0)
        if rc != 0:
            raise RuntimeError(f"axon_start_nrt_profile rc={rc}")
        try:
            yield
        finally:
            n = lib.axon_stop_nrt_profile(str(output_dir).encode())
            print(f"profile: {n} file(s) written to {output_dir}")

    return _hook


def run_traced(x, Wq, Wk, Wv, Wp, bp, outdir=None, cores=(0,)):
    """Run once under NTFF profiling; returns (out, exec_time_ns, trace_path)."""
    import glob
    import tempfile

    import gauge.profiler
    from concourse._compat import FishPath

    S = x.shape[1]
    fn, in_names, out_names, out_avals, mesh = _get_runner(S)
    in_maps = prep_inputs(x, Wq, Wk, Wv, Wp, bp)
    args = []
    for nm in in_names:
        if nm in _REPLICATED:
            args.append(in_maps[0][nm])
        else:
            args.append(np.concatenate([in_maps[c][nm] for c in range(NCORES)], axis=0))
    zero_outs = [
        np.zeros((NCORES * av.shape[0], *av.shape[1:]), av.dtype) for av in out_avals
    ]
    # warm (compile + first exec)
    out_arrs = fn(*args, *zero_outs)
    _ = [np.asarray(o) for o in out_arrs]

    if outdir is None:
        outdir = tempfile.mkdtemp(prefix="ntff_")
    hook = _ntff_hook()
    zero_outs = [
        np.zeros((NCORES * av.shape[0], *av.shape[1:]), av.dtype) for av in out_avals
    ]
    with hook(outdir, list(cores)):
        out_arrs = fn(*args, *zero_outs)
        out_np = [np.asarray(o) for o in out_arrs]

    ntffs = glob.glob(f"{outdir}/*.ntff")
    if not ntffs:
        print(f"no NTFF files in {outdir}")
        return None, None, None
    nc = get_nc(S)
    profile = gauge.profiler.Profile(
        profile_path=FishPath(outdir),
        kernel_dev_mode=True,
        profile_on_exit=False,
        bass_kernel=nc.m,
        offline_processing=True,
        fname="*_body*",
        metadata={"artifacts_path": outdir},
    )
    results = profile.to_perfetto(model_index=tuple(range(len(cores))))
    exec_ns = max(r.exec_time_ns for r in results)
    yfull = _assemble_y(
        [out_np[out_names.index("y")].reshape(NCORES, -1, D)[c] for c in range(NCORES)],
        S,
    )
    return yfull, exec_ns, results[0].trace_path


# revision 9
# speedup vs baseline: 4603.1200x; 4603.1200x over previous
"""Trainium2 Bass kernel for causal multi-head attention + output projection.

Problem: B=2, S=2048, D=1024, H=16 heads of HD=64; fp32; causal softmax
scaled by D**-0.5; output projection with bias.

Sharding: 2 heads per core (tensor parallel on heads) for QKV + attention.
Output rows are interleave-sharded at 64-column granularity (q-block g of 64
columns is owned by core g%8), so after EVERY 512-wide attention chunk the
cores exchange one small AllToAll ([8,128,64] bf16) instead of one big fp32
AllToAll per (batch, head-pair) at the end.  The producing core normalizes
(softmax denominator) and casts to bf16 BEFORE the exchange, so the
receiving core only runs output-projection matmuls.  This removes the
serial denominator/normalize chain from the kernel tail and overlaps all
communication with attention compute.

Math notes:
 - All attention tensors are kept transposed ([feature, seq] layouts) so
   every matmul contracts on the partition dim with zero on-chip transposes
   (except V, which is produced as V^T and transposed via the PE).
 - softmax is computed without max-subtraction: logits are N(0, 1/16) by
   construction, so exp() is numerically safe; the denominator is
   accumulated by a column of ones appended to V (row 64 of the O^T PSUM
   accumulator).
 - float32r (TF32-like) matmuls run at bf16 rate with ~1e-4 relative error.
"""

import sys

sys.path.insert(0, "/opt/trn_rl_repo")

import numpy as np

import concourse.bacc as bacc
import concourse.mybir as mybir
import concourse.tile as tile
from concourse.bass_utils import run_bass_kernel_spmd

B, D, H, HD = 2, 1024, 16, 64
NCORES = 8
SCALE = float(D) ** -0.5
F32 = mybir.dt.float32
F32R = mybir.dt.float32r
BF16 = mybir.dt.bfloat16
Exp = mybir.ActivationFunctionType.Exp


def build(S=2048, dump=False):
    KD = D // 128          # contraction tiles for the projections
    NT = S // 128          # key tiles
    SQ = 512               # query-chunk width
    NCH = S // SQ          # query chunks per (batch, head)
    SLOT = SQ // NCORES    # 64: per-core column slot inside a chunk
    NP = NCH // 2          # 128-row output "pairs" per batch per core
    YR = B * NP * 128      # output rows per core

    nc = bacc.Bacc("TRN2", target_bir_lowering=False, debug=False)
    xT = nc.dram_tensor("xT", [B, D, S], BF16, kind="ExternalInput")
    Wqkv = nc.dram_tensor("Wqkv", [128, 3, D // 128, 128], BF16, kind="ExternalInput")
    WpT = nc.dram_tensor("WpT", [128, D // 128, D], BF16, kind="ExternalInput")
    bp = nc.dram_tensor("bp", [1, D], BF16, kind="ExternalInput")
    mask = nc.dram_tensor("mask", [128, 128], BF16, kind="ExternalInput")
    idin = nc.dram_tensor("idin", [128, 128], BF16, kind="ExternalInput")
    # y rows (per core c): [b, p, j]: global q = SQ*(2p + j//64) + 64c + j%64
    y = nc.dram_tensor("y", [YR, D], F32, kind="ExternalOutput")

    with tile.TileContext(nc) as tc:
        ctx_pools = [
            tc.tile_pool(name="persist", bufs=1),
            tc.tile_pool(name="dram", bufs=1, space="DRAM"),
            tc.tile_pool(name="wq", bufs=1),
            tc.tile_pool(name="xp", bufs=2),
            tc.tile_pool(name="qk", bufs=2),
            tc.tile_pool(name="vp", bufs=2),
            tc.tile_pool(name="at", bufs=6),
            tc.tile_pool(name="fin", bufs=2),
            tc.tile_pool(name="yo", bufs=2),
            tc.tile_pool(name="ps_mix", bufs=2, space="PSUM"),
            tc.tile_pool(name="ps_sc", bufs=2, space="PSUM"),
            tc.tile_pool(name="ps_oT", bufs=2, space="PSUM"),
        ]
        import contextlib

        with contextlib.ExitStack() as stk:
            (
                persist, dram, wpool, xpool, qkpool, vppool, atpool,
                finpool, ypool, ps_mix, ps_sc, ps_oT,
            ) = [stk.enter_context(p) for p in ctx_pools]

            # ---- critical-path first: small constants, weights, batch-0 x ----
            ident = persist.tile([128, 128], BF16)
            nc.sync.dma_start(out=ident, in_=idin[:, :])
            mask_sb = persist.tile([128, 128], BF16)
            nc.sync.dma_start(out=mask_sb, in_=mask[:, :])
            wqkv_sb = wpool.tile([128, 3, KD, 128], BF16)
            nc.sync.dma_start(out=wqkv_sb, in_=Wqkv[:, :, :, :])
            # PE warm-up while input DMAs are in flight: ramps the clock gate
            wps = ps_sc.tile([128, 2, SQ], F32, tag="ps_sc", name="warmps")
            for _ in range(90):
                nc.tensor.matmul(wps[:, 0, 0:128], ident, ident, start=True, stop=True)

            def load_x(b):
                xs = [
                    xpool.tile([128, S], BF16, tag=f"x{t}", name=f"x_{b}_{t}")
                    for t in range(KD)
                ]
                for t in range(KD):
                    nc.sync.dma_start(
                        out=xs[t], in_=xT[b, 128 * t : 128 * (t + 1), :]
                    )
                return xs

            x_sb = {0: load_x(0)}

            ones_sb = persist.tile([1, 128], BF16)
            nc.vector.memset(ones_sb, 1.0)
            # all-ones row parked at partition 64 (to pair with PSUM row 64
            # of the O^T accumulator in the rcp-broadcast matmul)
            ones65 = persist.tile([65, 128], F32)
            nc.vector.memset(ones65[64:65, :], 1.0)

            # onrm[b]: [128 feat, KD sender, NCH chunk, SLOT] normalized bf16
            onrm = {
                b: persist.tile([128, KD, NCH, SLOT], BF16, name=f"onrm_{b}")
                for b in range(B)
            }
            a2a_in = {
                (b, n): dram.tile([NCORES, 2, 64, SLOT], BF16, name=f"a2a_in_{b}_{n}")
                for b in range(B)
                for n in range(NCH)
            }
            a2a_out = {
                (b, n): dram.tile([NCORES, 2, 64, SLOT], BF16, name=f"a2a_out_{b}_{n}")
                for b in range(B)
                for n in range(NCH)
            }
            qkvT = {}
            vp = {}
            ot_ps = {}     # (b, hs) -> live O^T psum tile of the current chunk
            fin_state = {}

            def emit_qkv_group(b, w, n, eng=None):
                if b not in qkvT:
                    qkvT[b] = qkpool.tile(
                        [128, 3, S], BF16, tag="qkvT", name=f"qkvT_{b}"
                    )
                pst = ps_mix.tile([128, SQ], F32, tag="mix", name=f"psqk_{b}_{w}_{n}")
                for t in range(KD):
                    nc.tensor.matmul(
                        pst,
                        wqkv_sb[:, w, t, :],
                        x_sb[b][t][:, SQ * n : SQ * (n + 1)],
                        start=(t == 0),
                        stop=(t == KD - 1),
                    )
                dst = qkvT[b][:, w, SQ * n : SQ * (n + 1)]
                if eng == "scalar":
                    nc.scalar.copy(dst, pst)
                else:
                    nc.vector.tensor_copy(dst, pst)

            def emit_v_unit(b, i):
                if b not in vp:
                    vp[b] = vppool.tile(
                        [128, NT, 2, 65], BF16, tag="vp", name=f"vp_{b}"
                    )
                    nc.vector.memset(vp[b][:, :, :, 64], 1.0)
                pst = ps_mix.tile([128, 128], BF16, tag="mix", name=f"psvt_{b}_{i}")
                nc.tensor.transpose(
                    pst, qkvT[b][:, 2, 128 * i : 128 * (i + 1)], ident[:, :]
                )
                for hs in range(2):
                    nc.vector.tensor_copy(
                        vp[b][:, i, hs, 0:64], pst[:, 64 * hs : 64 * hs + 64]
                    )

            def emit_attn_chunk(b, hs, n, fillers, early=None):
                qT = qkvT[b][64 * hs : 64 * hs + 64, 0, :]
                kT = qkvT[b][64 * hs : 64 * hs + 64, 1, :]
                ot = ps_oT.tile([65, SQ], F32, tag="ps_oT", name=f"ot_{b}_{hs}_{n}")
                ot_ps[(b, hs)] = ot
                jmax = 4 * n + 4
                for jp in range(0, jmax, 2):
                    sc = ps_sc.tile(
                        [128, 2, SQ], F32, tag="ps_sc", name=f"sc_{b}_{hs}_{n}_{jp}"
                    )
                    at = atpool.tile([128, 2, SQ], BF16, tag="at")
                    offs = []
                    for k in range(2):
                        j = jp + k
                        off = max(0, 128 * j - SQ * n)
                        offs.append(off)
                        nc.tensor.matmul(
                            sc[:, k, off:],
                            kT[:, 128 * j : 128 * (j + 1)],
                            qT[:, SQ * n + off : SQ * (n + 1)],
                            start=True,
                            stop=True,
                        )
                    # one exp over both halves (covers a dead zone between them)
                    o0 = offs[0]
                    nc.scalar.activation(
                        at[:, :, :].rearrange("p a s -> p (a s)")[:, o0:],
                        sc[:, :, :].rearrange("p a s -> p (a s)")[:, o0:],
                        Exp,
                        scale=SCALE,
                    )
                    for k in range(2):
                        j = jp + k
                        off = offs[k]
                        if j >= 4 * n:
                            nc.gpsimd.tensor_mul(
                                at[:, k, off : off + 128],
                                at[:, k, off : off + 128],
                                mask_sb,
                            )
                        nc.tensor.matmul(
                            ot[:, off:],
                            vp[b][:, j, hs, :],
                            at[:, k, off:],
                            start=(j == 0),
                            stop=(j == jmax - 1),
                        )
                    if early:
                        early.pop(0)()
                    elif fillers:
                        fillers[0] -= 2
                        if fillers[0] <= 0 and len(fillers) > 1:
                            fillers[0] = fillers.pop(1)
                            fillers.pop(1)()

            # ---- per-(b, n) finalize: normalize, pack bf16, exchange ----
            def emit_fin_rcp(b, n, hs):
                """Reciprocal of the denominator row (row 64 of the O^T psum).
                Emitted right after chunk (b, hs, n); non-PE ops only."""
                st = fin_state.setdefault((b, n), {})
                if "rcp" not in st:
                    st["rcp"] = finpool.tile(
                        [65, 2, SQ], F32R, tag="rcp", name=f"rcp_{b}_{n}"
                    )
                    st["bcs"] = finpool.tile(
                        [64, 2, SQ], F32, tag="bcs", name=f"bcs_{b}_{n}"
                    )
                    st["src"] = finpool.tile(
                        [64, 2, SQ], BF16, tag="src", name=f"src_{b}_{n}"
                    )
                st[f"ot{hs}"] = ot_ps[(b, hs)]
                with nc.allow_low_precision(reason="softmax denom recip"):
                    nc.vector.reciprocal(
                        st["rcp"][64:65, hs, :],
                        ot_ps[(b, hs)][64:65, :].bitcast(F32R),
                    )

            def emit_fin_norm(b, n, hs):
                """Broadcast 1/den over 64 partitions (PE outer product from
                partition 64 down to 0..63), then normalize+cast to bf16."""
                st = fin_state[(b, n)]
                bc = ps_mix.tile([64, SQ], F32, tag="mix", name=f"bc_{b}_{n}_{hs}")
                nc.tensor.matmul(
                    bc,
                    ones65[64:65, 0:64].bitcast(F32R),
                    st["rcp"][64:65, hs, :],
                    start=True,
                    stop=True,
                )
                nc.scalar.copy(st["bcs"][:, hs, :], bc)
                nc.vector.tensor_mul(
                    st["src"][:, hs, :],
                    st[f"ot{hs}"][0:64, :],
                    st["bcs"][:, hs, :],
                )

            def emit_fin_xchg(b, n):
                """Stage the normalized chunk into DRAM slots + AllToAll +
                unpack into the onrm projection operand."""
                st = fin_state[(b, n)]
                for d in range(NCORES):
                    for hs in range(2):
                        nc.sync.dma_start(
                            out=a2a_in[(b, n)][d, hs, :, :],
                            in_=st["src"][:, hs, SLOT * d : SLOT * (d + 1)],
                        )
                nc.gpsimd.collective_compute(
                    "AllToAll",
                    mybir.AluOpType.bypass,
                    replica_groups=[list(range(NCORES))],
                    ins=[a2a_in[(b, n)][:, :, :, :].opt()],
                    outs=[a2a_out[(b, n)][:, :, :, :].opt()],
                )
                # unpack: sender t -> contraction block t of onrm
                nc.sync.dma_start(
                    out=onrm[b][:, :, n, :],
                    in_=a2a_out[(b, n)][:, :, :, :].rearrange("d h p c -> (h p) d c"),
                )

            # ---- output projection for one 128-row pair (chunks 2p, 2p+1) ----
            ydict = {}

            def emit_D_group(b, p, nn):
                acc = ps_mix.tile([128, 512], F32, tag="mix", name=f"acc_{b}_{p}_{nn}")
                for t in range(KD):
                    nc.tensor.matmul(
                        acc,
                        onrm[b][:, t, 2 * p : 2 * p + 2, :],
                        wpT_sb[:, t, 512 * nn : 512 * (nn + 1)],
                        start=(t == 0),
                        stop=False,
                    )
                nc.tensor.matmul(
                    acc, ones_sb, bp_sb[:, 512 * nn : 512 * (nn + 1)],
                    start=False, stop=True,
                )
                if (b, p) not in ydict:
                    ydict[(b, p)] = ypool.tile([128, D], F32, tag="y", name=f"y_{b}_{p}")
                ys = ydict[(b, p)]
                nc.vector.tensor_copy(ys[:, 512 * nn : 512 * (nn + 1)], acc)
                if nn == D // 512 - 1:
                    r0 = (b * NP + p) * 128
                    nc.sync.dma_start(out=y[r0 : r0 + 128, :], in_=ys)

            # ---- schedule ----
            # batch 0 projections (dense PE, warms HAM)
            for w in range(3):
                for n in range(NCH):
                    emit_qkv_group(0, w, n, eng="scalar")
            for i in range(NT):
                emit_v_unit(0, i)
            # batch-1 x load starts now (behind batch-0 x on the DMA queues)
            x_sb[1] = load_x(1)
            # deferred big loads for the output projection
            wpT_sb = persist.tile([128, KD, D], BF16)
            nc.sync.dma_start(out=wpT_sb, in_=WpT[:, :, :])
            bp_sb = persist.tile([1, D], BF16)
            nc.sync.dma_start(out=bp_sb, in_=bp[:, :])

            # ---- filler plan ----
            # linear chunk order; chunk ci=(b,n) gets woven-in units:
            #  - tail of the previous chunk (hs1 normalize + exchange)
            #  - its static unit list (b1 prep during b0, D groups later)
            chunks = [(b, n) for b in range(B) for n in range(NCH)]
            upc = {ci: [] for ci in chunks}

            b1_units = []
            for w in range(3):
                for n in range(NCH):
                    b1_units.append(lambda w=w, n=n: emit_qkv_group(1, w, n))
            for i in range(NT):
                b1_units.append(lambda i=i: emit_v_unit(1, i))
            # distribute b1 prep units over batch-0 chunks, weighted by size
            w0 = [4 * n + 4 for n in range(NCH)]
            tot0 = sum(w0)
            pos = 0
            for n in range(NCH):
                take = max(1, round(len(b1_units) * w0[n] / tot0))
                if n == NCH - 1:
                    take = len(b1_units) - pos
                upc[(0, n)].extend(b1_units[pos : pos + take])
                pos += take

            # D(b, p) needs the exchange of chunks (b,2p) and (b,2p+1); the
            # exchange of (b,m) is emitted at the START of chunk index
            # idx(b,m)+1, so D units are safe from chunk index idx(b,2p+1)+2.
            tail_d = []
            for b in range(B):
                for p in range(NP):
                    idx = b * NCH + (2 * p + 1) + 2
                    units = [
                        lambda b=b, p=p, nn=nn: emit_D_group(b, p, nn)
                        for nn in range(D // 512)
                    ]
                    if idx < len(chunks):
                        upc[chunks[idx]].extend(units)
                    else:
                        tail_d.extend(units)

            gf = [4]  # global filler list: [countdown, unit, countdown, ...]

            def gf_add(units, total_j):
                pace = max(2, total_j // max(len(units), 1))
                for u in units:
                    gf.extend([pace, u])

            prev = None
            for (b, n) in chunks:
                if (b, n) == (1, 0):
                    # all batch-1 prep (qkv/v) must be emitted before batch-1
                    # attention reads it: force-drain leftover fillers
                    while len(gf) > 1:
                        gf.pop(1)
                        gf.pop(1)()
                    gf[:] = [4]
                early0 = []
                if prev is not None:
                    pb, pn = prev

                    def prev_tail(pb=pb, pn=pn):
                        emit_fin_norm(pb, pn, 1)
                        emit_fin_xchg(pb, pn)

                    early0.append(prev_tail)
                total_j = 2 * (4 * n + 4)
                gf_add(upc[(b, n)], total_j)
                emit_attn_chunk(b, 0, n, gf, early=early0)
                emit_fin_rcp(b, n, 0)
                # hs0's normalize runs early inside the hs1 chunk
                early1 = [lambda b=b, n=n: emit_fin_norm(b, n, 0)]
                emit_attn_chunk(b, 1, n, gf, early=early1)
                emit_fin_rcp(b, n, 1)
                for u in early0 + early1:
                    u()  # (n=0 chunks have >=2 jp slots, so normally empty)
                prev = (b, n)

            # drain leftover fillers
            while len(gf) > 1:
                gf.pop(1)
                gf.pop(1)()
            # tail: last chunk's normalize + exchange + final D groups
            emit_fin_norm(1, NCH - 1, 1)
            emit_fin_xchg(1, NCH - 1)
            for u in tail_d:
                u()

    nc.compile()
    return nc


_built = {}


def get_nc(S=2048):
    if S not in _built:
        _built[S] = build(S)
    return _built[S]


def prep_inputs(x, Wq, Wk, Wv, Wp, bp):
    """Host-side shard prep. Returns per-core input maps."""
    import ml_dtypes

    BF = ml_dtypes.bfloat16
    x = np.ascontiguousarray(np.asarray(x, dtype=np.float32))
    Wq, Wk, Wv = (np.asarray(w, dtype=np.float32) for w in (Wq, Wk, Wv))
    Wp = np.asarray(Wp, dtype=np.float32)
    bp = np.asarray(bp, dtype=np.float32)
    BFc = BF
    xT = np.ascontiguousarray(x.transpose(0, 2, 1)).astype(BFc)
    KD = D // 128
    # WpT pre-arranged for SBUF: [p, t, i] with row t*128+p of Wp.T
    WpT = np.ascontiguousarray(
        Wp.T.reshape(KD, 128, D).transpose(1, 0, 2)
    ).astype(BFc)
    mask = np.triu(np.ones((128, 128), dtype=np.float32)).astype(BFc)
    idin = np.eye(128, dtype=np.float32).astype(BFc)
    in_maps = []
    for c in range(NCORES):
        h0 = 2 * c
        wqkv = np.stack(
            [
                np.concatenate([Wq[h0], Wq[h0 + 1]], axis=1),
                np.concatenate([Wk[h0], Wk[h0 + 1]], axis=1),
                np.concatenate([Wv[h0], Wv[h0 + 1]], axis=1),
            ]
        )  # [3, D, 128]
        # pre-arrange: [p, w, t, m]
        wqkv = np.ascontiguousarray(
            wqkv.reshape(3, KD, 128, 128).transpose(2, 0, 1, 3)
        ).astype(BF)
        in_maps.append(
            {
                "xT": xT,
                "Wqkv": wqkv,
                "WpT": WpT,
                "bp": bp.reshape(1, D).astype(BF),
                "mask": mask,
                "idin": idin,
            }
        )
    return in_maps


# inputs identical across cores are passed replicated (shipped once, not 8x)
_REPLICATED = {"xT", "WpT", "bp", "mask", "idin"}

_runners = {}


def _get_runner(S):
    """Cached jitted SPMD callable for the built module."""
    if S in _runners:
        return _runners[S]
    import jax
    import concourse.mybir as _mybir
    from concourse import bass2jax
    from jax.experimental.shard_map import shard_map
    from jax.sharding import Mesh, PartitionSpec

    nc = get_nc(S)
    bass2jax.install_neuronx_cc_hook()

    in_names, out_names, out_avals = [], [], []
    partition_name = nc.partition_id_tensor.name if nc.partition_id_tensor else None
    for alloc in nc.m.functions[0].allocations:
        if not isinstance(alloc, _mybir.MemoryLocationSet):
            continue
        name = alloc.memorylocations[0].name
        if alloc.kind == "ExternalInput":
            if name != partition_name:
                in_names.append(name)
        elif alloc.kind == "ExternalOutput":
            out_names.append(name)
            out_avals.append(
                jax.core.ShapedArray(tuple(alloc.tensor_shape), _mybir.dt.np(alloc.dtype))
            )
    n_params = len(in_names)
    all_in_names = list(in_names) + list(out_names)
    if partition_name is not None:
        all_in_names.append(partition_name)

    def _body(*args):
        operands = list(args)
        if partition_name is not None:
            operands.append(bass2jax.partition_id_tensor())
        outs = bass2jax._bass_exec_p.bind(
            *operands,
            out_avals=tuple(out_avals),
            in_names=tuple(all_in_names),
            out_names=tuple(out_names),
            lowering_input_output_aliases=(),
            sim_require_finite=True,
            sim_require_nnan=True,
            nc=nc,
        )
        return tuple(outs)

    devices = jax.devices()[:NCORES]
    mesh = Mesh(np.asarray(devices), ("core",))
    in_specs = tuple(
        PartitionSpec() if nm in _REPLICATED else PartitionSpec("core")
        for nm in in_names
    ) + (PartitionSpec("core"),) * len(out_names)
    out_specs = (PartitionSpec("core"),) * len(out_names)
    donate = tuple(range(n_params, n_params + len(out_names)))
    fn = jax.jit(
        shard_map(_body, mesh=mesh, in_specs=in_specs, out_specs=out_specs, check_rep=False),
        donate_argnums=donate,
        keep_unused=True,
    )
    r = (fn, in_names, out_names, out_avals, mesh)
    _runners[S] = r
    return r


class _Res:
    def __init__(self, results):
        self.results = results
        self.exec_time_ns = None


def run(x, Wq, Wk, Wv, Wp, bp, timings=None):
    import time as _time

    S = x.shape[1]
    t0 = _time.perf_counter()
    fn, in_names, out_names, out_avals, mesh = _get_runner(S)
    t1 = _time.perf_counter()
    in_maps = prep_inputs(x, Wq, Wk, Wv, Wp, bp)
    t2 = _time.perf_counter()
    args = []
    for nm in in_names:
        if nm in _REPLICATED:
            args.append(in_maps[0][nm])
        else:
            args.append(np.concatenate([in_maps[c][nm] for c in range(NCORES)], axis=0))
    zero_outs = [
        np.zeros((NCORES * av.shape[0], *av.shape[1:]), av.dtype) for av in out_avals
    ]
    t3 = _time.perf_counter()
    out_arrs = fn(*args, *zero_outs)
    out_np = [np.asarray(o) for o in out_arrs]
    t4 = _time.perf_counter()
    results = [
        {
            nm: out_np[i].reshape(NCORES, *out_avals[i].shape)[c]
            for i, nm in enumerate(out_names)
        }
        for c in range(NCORES)
    ]
    if timings is not None:
        timings.update(
            runner=t1 - t0, prep=t2 - t1, concat=t3 - t2, exec=t4 - t3
        )
    return _assemble_y([results[c]["y"] for c in range(NCORES)], S), _Res(results)


def _assemble_y(per_core, S):
    """per-core y rows [b, p, j]: global q = SQ*(2p + j//64) + 64c + j%64."""
    SQ = 512
    NCH = S // SQ
    NP = NCH // 2
    out = np.empty((B, S, D), dtype=per_core[0].dtype)
    for c in range(NCORES):
        yc = per_core[c].reshape(B, NP, 128, D)
        for b in range(B):
            for p in range(NP):
                for half in range(2):
                    q0 = SQ * (2 * p + half) + 64 * c
                    out[b, q0 : q0 + 64, :] = yc[b, p, 64 * half : 64 * half + 64]
    return out


def kernel(x, Wq, Wk, Wv, Wp, bp):
    out, _ = run(x, Wq, Wk, Wv, Wp, bp)
    return out


# ---------------------------------------------------------------------------
# NTFF profiling support (test harness only; not needed for kernel()).
# ---------------------------------------------------------------------------

def _ntff_hook():
    import contextlib
    import ctypes

    lib = ctypes.CDLL("/opt/axon/libaxon_pjrt.so")
    lib.axon_start_nrt_profile.argtypes = [
        ctypes.POINTER(ctypes.c_int64),
        ctypes.c_size_t,
    ]
    lib.axon_start_nrt_profile.restype = ctypes.c_int64
    lib.axon_stop_nrt_profile.argtypes = [ctypes.c_char_p]
    lib.axon_stop_nrt_profile.restype = ctypes.c_int64

    @contextlib.contextmanager
    def _hook(output_dir, device_ids):
        import jax

        jax.devices()
        if device_ids:
            ids = (ctypes.c_int64 * len(device_ids))(*device_ids)
            rc = lib.axon_start_nrt_profile(ids, len(device_ids))
        else:
            rc = lib.axon_start_nrt_profile(None, 0)
        if rc != 0:
            raise RuntimeError(f"axon_start_nrt_profile rc={rc}")
        try:
            yield
        finally:
            n = lib.axon_stop_nrt_profile(str(output_dir).encode())
            print(f"profile: {n} file(s) written to {output_dir}")

    return _hook


def run_traced(x, Wq, Wk, Wv, Wp, bp, outdir=None, cores=(0,)):
    """Run once under NTFF profiling; returns (out, exec_time_ns, trace_path)."""
    import glob
    import tempfile

    import gauge.profiler
    from concourse._compat import FishPath

    S = x.shape[1]
    fn, in_names, out_names, out_avals, mesh = _get_runner(S)
    in_maps = prep_inputs(x, Wq, Wk, Wv, Wp, bp)
    args = []
    for nm in in_names:
        if nm in _REPLICATED:
            args.append(in_maps[0][nm])
        else:
            args.append(np.concatenate([in_maps[c][nm] for c in range(NCORES)], axis=0))
    zero_outs = [
        np.zeros((NCORES * av.shape[0], *av.shape[1:]), av.dtype) for av in out_avals
    ]
    # warm (compile + first exec)
    out_arrs = fn(*args, *zero_outs)
    _ = [np.asarray(o) for o in out_arrs]

    if outdir is None:
        outdir = tempfile.mkdtemp(prefix="ntff_")
    hook = _ntff_hook()
    zero_outs = [
        np.zeros((NCORES * av.shape[0], *av.shape[1:]), av.dtype) for av in out_avals
    ]
    with hook(outdir, list(cores)):
        out_arrs = fn(*args, *zero_outs)
        out_np = [np.asarray(o) for o in out_arrs]

    ntffs = glob.glob(f"{outdir}/*.ntff")
    if not ntffs:
        print(f"no NTFF files in {outdir}")
        return None, None, None
    nc = get_nc(S)
    profile = gauge.profiler.Profile(
        profile_path=FishPath(outdir),
        kernel_dev_mode=True,
        profile_on_exit=False,
        bass_kernel=nc.m,
        offline_processing=True,
        fname="*_body*",
        metadata={"artifacts_path": outdir},
    )
    results = profile.to_perfetto(model_index=tuple(range(len(cores))))
    exec_ns = max(r.exec_time_ns for r in results)
    yfull = _assemble_y(
        [out_np[out_names.index("y")].reshape(NCORES, -1, D)[c] for c in range(NCORES)],
        S,
    )
    return yfull, exec_ns, results[0].trace_path


# revision 10
# speedup vs baseline: 4810.4836x; 1.0450x over previous
"""Trainium2 Bass kernel for causal multi-head attention + output projection.

Problem: B=2, S=2048, D=1024, H=16 heads of HD=64; fp32; causal softmax
scaled by D**-0.5; output projection with bias.

Sharding: 2 heads per core (tensor parallel on heads) for QKV + attention.
Output rows are interleave-sharded at 64-column granularity (q-block g of 64
columns is owned by core g%8), so after EVERY 512-wide attention chunk the
cores exchange one small AllToAll ([8,128,64] bf16) instead of one big fp32
AllToAll per (batch, head-pair) at the end.  The producing core normalizes
(softmax denominator) and casts to bf16 BEFORE the exchange, so the
receiving core only runs output-projection matmuls.  This removes the
serial denominator/normalize chain from the kernel tail and overlaps all
communication with attention compute.

Math notes:
 - All attention tensors are kept transposed ([feature, seq] layouts) so
   every matmul contracts on the partition dim with zero on-chip transposes
   (except V, which is produced as V^T and transposed via the PE).
 - softmax is computed without max-subtraction: logits are N(0, 1/16) by
   construction, so exp() is numerically safe; the denominator is
   accumulated by a column of ones appended to V (row 64 of the O^T PSUM
   accumulator).
 - float32r (TF32-like) matmuls run at bf16 rate with ~1e-4 relative error.
"""

import sys

sys.path.insert(0, "/opt/trn_rl_repo")

import numpy as np

import concourse.bacc as bacc
import concourse.mybir as mybir
import concourse.tile as tile
from concourse.bass_utils import run_bass_kernel_spmd

B, D, H, HD = 2, 1024, 16, 64
NCORES = 8
SCALE = float(D) ** -0.5
F32 = mybir.dt.float32
F32R = mybir.dt.float32r
BF16 = mybir.dt.bfloat16
Exp = mybir.ActivationFunctionType.Exp


def build(S=2048, dump=False):
    KD = D // 128          # contraction tiles for the projections
    NT = S // 128          # key tiles
    SQ = 512               # query-chunk width
    NCH = S // SQ          # query chunks per (batch, head)
    SLOT = SQ // NCORES    # 64: per-core column slot inside a chunk
    NP = NCH // 2          # 128-row output "pairs" per batch per core
    YR = B * NP * 128      # output rows per core

    nc = bacc.Bacc("TRN2", target_bir_lowering=False, debug=False)
    xT = nc.dram_tensor("xT", [B, D, S], BF16, kind="ExternalInput")
    Wqkv = nc.dram_tensor("Wqkv", [128, 3, D // 128, 128], BF16, kind="ExternalInput")
    WpT = nc.dram_tensor("WpT", [128, D // 128, D], BF16, kind="ExternalInput")
    bp = nc.dram_tensor("bp", [1, D], BF16, kind="ExternalInput")
    mask = nc.dram_tensor("mask", [128, 128], BF16, kind="ExternalInput")
    idin = nc.dram_tensor("idin", [128, 128], BF16, kind="ExternalInput")
    # y rows (per core c): [b, p, j]: global q = SQ*(2p + j//64) + 64c + j%64
    y = nc.dram_tensor("y", [YR, D], F32, kind="ExternalOutput")

    with tile.TileContext(nc) as tc:
        ctx_pools = [
            tc.tile_pool(name="persist", bufs=1),
            tc.tile_pool(name="dram", bufs=1, space="DRAM"),
            tc.tile_pool(name="wq", bufs=1),
            tc.tile_pool(name="xp", bufs=2),
            tc.tile_pool(name="qk", bufs=2),
            tc.tile_pool(name="vp", bufs=2),
            tc.tile_pool(name="at", bufs=6),
            tc.tile_pool(name="fin", bufs=2),
            tc.tile_pool(name="yo", bufs=2),
            tc.tile_pool(name="ps_mix", bufs=2, space="PSUM"),
            tc.tile_pool(name="ps_sc", bufs=2, space="PSUM"),
            tc.tile_pool(name="ps_oT", bufs=2, space="PSUM"),
        ]
        import contextlib

        with contextlib.ExitStack() as stk:
            (
                persist, dram, wpool, xpool, qkpool, vppool, atpool,
                finpool, ypool, ps_mix, ps_sc, ps_oT,
            ) = [stk.enter_context(p) for p in ctx_pools]

            # ---- critical-path first: small constants, weights, batch-0 x ----
            ident = persist.tile([128, 128], BF16)
            nc.sync.dma_start(out=ident, in_=idin[:, :])
            mask_sb = persist.tile([128, 128], BF16)
            nc.sync.dma_start(out=mask_sb, in_=mask[:, :])
            wqkv_sb = wpool.tile([128, 3, KD, 128], BF16)
            nc.sync.dma_start(out=wqkv_sb, in_=Wqkv[:, :, :, :])
            # PE warm-up while input DMAs are in flight: ramps the clock gate
            wps = ps_sc.tile([128, 2, SQ], F32, tag="ps_sc", name="warmps")
            for _ in range(90):
                nc.tensor.matmul(wps[:, 0, 0:128], ident, ident, start=True, stop=True)

            def load_x(b):
                xs = [
                    xpool.tile([128, S], BF16, tag=f"x{t}", name=f"x_{b}_{t}")
                    for t in range(KD)
                ]
                for t in range(KD):
                    nc.sync.dma_start(
                        out=xs[t], in_=xT[b, 128 * t : 128 * (t + 1), :]
                    )
                return xs

            x_sb = {0: load_x(0)}

            ones_sb = persist.tile([1, 128], BF16)
            nc.vector.memset(ones_sb, 1.0)
            # all-ones row parked at partition 64 (to pair with PSUM row 64
            # of the O^T accumulator in the rcp-broadcast matmul)
            ones65 = persist.tile([65, 128], F32)
            nc.vector.memset(ones65[64:65, :], 1.0)

            # onrm[b]: [128 feat, KD sender, NCH chunk, SLOT] normalized bf16
            onrm = {
                b: persist.tile([128, KD, NCH, SLOT], BF16, name=f"onrm_{b}")
                for b in range(B)
            }
            a2a_in = {
                (b, p): dram.tile(
                    [NCORES, 2, 64, 2, SLOT], BF16, name=f"a2a_in_{b}_{p}"
                )
                for b in range(B)
                for p in range(NP)
            }
            a2a_out = {
                (b, p): dram.tile(
                    [NCORES, 2, 64, 2, SLOT], BF16, name=f"a2a_out_{b}_{p}"
                )
                for b in range(B)
                for p in range(NP)
            }
            qkvT = {}
            vp = {}
            ot_ps = {}     # (b, hs) -> live O^T psum tile of the current chunk
            fin_state = {}

            def emit_qkv_group(b, w, n, eng=None):
                if b not in qkvT:
                    qkvT[b] = qkpool.tile(
                        [128, 3, S], BF16, tag="qkvT", name=f"qkvT_{b}"
                    )
                pst = ps_mix.tile([128, SQ], F32, tag="mix", name=f"psqk_{b}_{w}_{n}")
                for t in range(KD):
                    nc.tensor.matmul(
                        pst,
                        wqkv_sb[:, w, t, :],
                        x_sb[b][t][:, SQ * n : SQ * (n + 1)],
                        start=(t == 0),
                        stop=(t == KD - 1),
                    )
                dst = qkvT[b][:, w, SQ * n : SQ * (n + 1)]
                if eng == "scalar":
                    nc.scalar.copy(dst, pst)
                else:
                    nc.vector.tensor_copy(dst, pst)

            def emit_v_unit(b, i):
                if b not in vp:
                    vp[b] = vppool.tile(
                        [128, NT, 2, 65], BF16, tag="vp", name=f"vp_{b}"
                    )
                    nc.vector.memset(vp[b][:, :, :, 64], 1.0)
                pst = ps_mix.tile([128, 128], BF16, tag="mix", name=f"psvt_{b}_{i}")
                nc.tensor.transpose(
                    pst, qkvT[b][:, 2, 128 * i : 128 * (i + 1)], ident[:, :]
                )
                for hs in range(2):
                    nc.vector.tensor_copy(
                        vp[b][:, i, hs, 0:64], pst[:, 64 * hs : 64 * hs + 64]
                    )

            def emit_attn_chunk(b, hs, n, fillers, early=None):
                qT = qkvT[b][64 * hs : 64 * hs + 64, 0, :]
                kT = qkvT[b][64 * hs : 64 * hs + 64, 1, :]
                ot = ps_oT.tile([65, SQ], F32, tag="ps_oT", name=f"ot_{b}_{hs}_{n}")
                ot_ps[(b, hs)] = ot
                jmax = 4 * n + 4
                for jp in range(0, jmax, 2):
                    sc = ps_sc.tile(
                        [128, 2, SQ], F32, tag="ps_sc", name=f"sc_{b}_{hs}_{n}_{jp}"
                    )
                    at = atpool.tile([128, 2, SQ], BF16, tag="at")
                    offs = []
                    for k in range(2):
                        j = jp + k
                        off = max(0, 128 * j - SQ * n)
                        offs.append(off)
                        nc.tensor.matmul(
                            sc[:, k, off:],
                            kT[:, 128 * j : 128 * (j + 1)],
                            qT[:, SQ * n + off : SQ * (n + 1)],
                            start=True,
                            stop=True,
                        )
                    # one exp over both halves (covers a dead zone between them)
                    o0 = offs[0]
                    nc.scalar.activation(
                        at[:, :, :].rearrange("p a s -> p (a s)")[:, o0:],
                        sc[:, :, :].rearrange("p a s -> p (a s)")[:, o0:],
                        Exp,
                        scale=SCALE,
                    )
                    for k in range(2):
                        j = jp + k
                        off = offs[k]
                        if j >= 4 * n:
                            nc.gpsimd.tensor_mul(
                                at[:, k, off : off + 128],
                                at[:, k, off : off + 128],
                                mask_sb,
                            )
                        nc.tensor.matmul(
                            ot[:, off:],
                            vp[b][:, j, hs, :],
                            at[:, k, off:],
                            start=(j == 0),
                            stop=(j == jmax - 1),
                        )
                    if early is not None and jp >= 2 and early:
                        early.pop(0)()
                    elif fillers:
                        fillers[0] -= 2
                        if fillers[0] <= 0 and len(fillers) > 1:
                            fillers[0] = fillers.pop(1)
                            fillers.pop(1)()

            # ---- per-(b, n) finalize: normalize, pack bf16, exchange ----
            def emit_fin_rcp(b, n, hs):
                """Reciprocal of the denominator row (row 64 of the O^T psum).
                Emitted right after chunk (b, hs, n); non-PE ops only."""
                st = fin_state.setdefault((b, n), {})
                if "rcp" not in st:
                    st["rcp"] = finpool.tile(
                        [65, 2, SQ], F32R, tag="rcp", name=f"rcp_{b}_{n}"
                    )
                    st["bcs"] = finpool.tile(
                        [64, 2, SQ], F32, tag="bcs", name=f"bcs_{b}_{n}"
                    )
                    st["src"] = finpool.tile(
                        [64, 2, SQ], BF16, tag="src", name=f"src_{b}_{n}"
                    )
                st[f"ot{hs}"] = ot_ps[(b, hs)]
                with nc.allow_low_precision(reason="softmax denom recip"):
                    nc.vector.reciprocal(
                        st["rcp"][64:65, hs, :],
                        ot_ps[(b, hs)][64:65, :].bitcast(F32R),
                    )

            def emit_fin_norm(b, n, hs):
                """Broadcast 1/den over 64 partitions (PE outer product from
                partition 64 down to 0..63), then normalize+cast to bf16."""
                st = fin_state[(b, n)]
                bc = ps_mix.tile([64, SQ], F32, tag="mix", name=f"bc_{b}_{n}_{hs}")
                nc.tensor.matmul(
                    bc,
                    ones65[64:65, 0:64].bitcast(F32R),
                    st["rcp"][64:65, hs, :],
                    start=True,
                    stop=True,
                )
                nc.scalar.copy(st["bcs"][:, hs, :], bc)
                nc.vector.tensor_mul(
                    st["src"][:, hs, :],
                    st[f"ot{hs}"][0:64, :],
                    st["bcs"][:, hs, :],
                )

            def emit_fin_stage(b, n):
                """Stage the normalized chunk into its pair's DRAM slots."""
                st = fin_state[(b, n)]
                p, cn = n // 2, n % 2
                for hs in range(2):
                    nc.sync.dma_start(
                        out=a2a_in[(b, p)][:, hs, :, cn, :].rearrange(
                            "d pp c -> pp d c"
                        ),
                        in_=st["src"][:, hs, :],
                    )

            def emit_pair_a2a(b, p):
                """AllToAll one chunk pair + unpack into onrm."""
                nc.gpsimd.collective_compute(
                    "AllToAll",
                    mybir.AluOpType.bypass,
                    replica_groups=[list(range(NCORES))],
                    ins=[a2a_in[(b, p)][:, :, :, :, :].opt()],
                    outs=[a2a_out[(b, p)][:, :, :, :, :].opt()],
                )
                nc.sync.dma_start(
                    out=onrm[b][:, :, 2 * p : 2 * p + 2, :],
                    in_=a2a_out[(b, p)][:, :, :, :, :].rearrange(
                        "d h pp cn c -> (h pp) d cn c"
                    ),
                )

            # ---- output projection for one 128-row pair (chunks 2p, 2p+1) ----
            ydict = {}

            def emit_D_group(b, p, nn):
                acc = ps_mix.tile([128, 512], F32, tag="mix", name=f"acc_{b}_{p}_{nn}")
                for t in range(KD):
                    nc.tensor.matmul(
                        acc,
                        onrm[b][:, t, 2 * p : 2 * p + 2, :],
                        wpT_sb[:, t, 512 * nn : 512 * (nn + 1)],
                        start=(t == 0),
                        stop=False,
                    )
                nc.tensor.matmul(
                    acc, ones_sb, bp_sb[:, 512 * nn : 512 * (nn + 1)],
                    start=False, stop=True,
                )
                if (b, p) not in ydict:
                    ydict[(b, p)] = ypool.tile([128, D], F32, tag="y", name=f"y_{b}_{p}")
                ys = ydict[(b, p)]
                nc.vector.tensor_copy(ys[:, 512 * nn : 512 * (nn + 1)], acc)
                if nn == D // 512 - 1:
                    r0 = (b * NP + p) * 128
                    nc.sync.dma_start(out=y[r0 : r0 + 128, :], in_=ys)

            # ---- schedule ----
            # batch 0 projections (dense PE, warms HAM)
            for w in range(3):
                for n in range(NCH):
                    emit_qkv_group(0, w, n, eng="scalar")
            for i in range(NT):
                emit_v_unit(0, i)
            # batch-1 x load starts now (behind batch-0 x on the DMA queues)
            x_sb[1] = load_x(1)
            # deferred big loads for the output projection
            wpT_sb = persist.tile([128, KD, D], BF16)
            nc.sync.dma_start(out=wpT_sb, in_=WpT[:, :, :])
            bp_sb = persist.tile([1, D], BF16)
            nc.sync.dma_start(out=bp_sb, in_=bp[:, :])

            # ---- filler plan ----
            # linear chunk order; chunk ci=(b,n) gets woven-in units:
            #  - tail of the previous chunk (hs1 normalize + exchange)
            #  - its static unit list (b1 prep during b0, D groups later)
            chunks = [(b, n) for b in range(B) for n in range(NCH)]
            upc = {ci: [] for ci in chunks}

            b1_units = []
            for w in range(3):
                for n in range(NCH):
                    b1_units.append(lambda w=w, n=n: emit_qkv_group(1, w, n))
            for i in range(NT):
                b1_units.append(lambda i=i: emit_v_unit(1, i))
            # distribute b1 prep units over batch-0 chunks, weighted by size
            w0 = [4 * n + 4 for n in range(NCH)]
            tot0 = sum(w0)
            pos = 0
            for n in range(NCH):
                take = max(1, round(len(b1_units) * w0[n] / tot0))
                if n == NCH - 1:
                    take = len(b1_units) - pos
                upc[(0, n)].extend(b1_units[pos : pos + take])
                pos += take

            # D(b, p) needs the exchange of chunks (b,2p) and (b,2p+1); the
            # exchange of (b,m) is emitted at the START of chunk index
            # idx(b,m)+1, so D units are safe from chunk index idx(b,2p+1)+2.
            tail_d = []
            for b in range(B):
                for p in range(NP):
                    idx = b * NCH + (2 * p + 1) + 2
                    units = [
                        lambda b=b, p=p, nn=nn: emit_D_group(b, p, nn)
                        for nn in range(D // 512)
                    ]
                    if idx < len(chunks):
                        upc[chunks[idx]].extend(units)
                    else:
                        tail_d.extend(units)

            gf = [4]  # global filler list: [countdown, unit, countdown, ...]

            def gf_add(units, total_j):
                pace = max(2, total_j // max(len(units), 1))
                for u in units:
                    gf.extend([pace, u])

            prev = None
            for (b, n) in chunks:
                if (b, n) == (1, 0):
                    # all batch-1 prep (qkv/v) must be emitted before batch-1
                    # attention reads it: force-drain leftover fillers
                    while len(gf) > 1:
                        gf.pop(1)
                        gf.pop(1)()
                    gf[:] = [4]
                early0 = []
                if prev is not None:
                    pb, pn = prev

                    def prev_tail(pb=pb, pn=pn):
                        emit_fin_norm(pb, pn, 1)
                        emit_fin_stage(pb, pn)
                        if pn % 2 == 1:
                            emit_pair_a2a(pb, pn // 2)

                    early0.append(prev_tail)
                total_j = 2 * (4 * n + 4)
                gf_add(upc[(b, n)], total_j)
                emit_attn_chunk(b, 0, n, gf, early=early0)
                emit_fin_rcp(b, n, 0)
                # hs0's normalize runs early inside the hs1 chunk
                early1 = [lambda b=b, n=n: emit_fin_norm(b, n, 0)]
                emit_attn_chunk(b, 1, n, gf, early=early1)
                emit_fin_rcp(b, n, 1)
                for u in early0 + early1:
                    u()  # (n=0 chunks have >=2 jp slots, so normally empty)
                prev = (b, n)

            # drain leftover fillers
            while len(gf) > 1:
                gf.pop(1)
                gf.pop(1)()
            # tail: last chunk's normalize + exchange + final D groups
            emit_fin_norm(1, NCH - 1, 1)
            emit_fin_stage(1, NCH - 1)
            emit_pair_a2a(1, NP - 1)
            for u in tail_d:
                u()

    nc.compile()
    return nc


_built = {}


def get_nc(S=2048):
    if S not in _built:
        _built[S] = build(S)
    return _built[S]


def prep_inputs(x, Wq, Wk, Wv, Wp, bp):
    """Host-side shard prep. Returns per-core input maps."""
    import ml_dtypes

    BF = ml_dtypes.bfloat16
    x = np.ascontiguousarray(np.asarray(x, dtype=np.float32))
    Wq, Wk, Wv = (np.asarray(w, dtype=np.float32) for w in (Wq, Wk, Wv))
    Wp = np.asarray(Wp, dtype=np.float32)
    bp = np.asarray(bp, dtype=np.float32)
    BFc = BF
    xT = np.ascontiguousarray(x.transpose(0, 2, 1)).astype(BFc)
    KD = D // 128
    # WpT pre-arranged for SBUF: [p, t, i] with row t*128+p of Wp.T
    WpT = np.ascontiguousarray(
        Wp.T.reshape(KD, 128, D).transpose(1, 0, 2)
    ).astype(BFc)
    mask = np.triu(np.ones((128, 128), dtype=np.float32)).astype(BFc)
    idin = np.eye(128, dtype=np.float32).astype(BFc)
    in_maps = []
    for c in range(NCORES):
        h0 = 2 * c
        wqkv = np.stack(
            [
                np.concatenate([Wq[h0], Wq[h0 + 1]], axis=1),
                np.concatenate([Wk[h0], Wk[h0 + 1]], axis=1),
                np.concatenate([Wv[h0], Wv[h0 + 1]], axis=1),
            ]
        )  # [3, D, 128]
        # pre-arrange: [p, w, t, m]
        wqkv = np.ascontiguousarray(
            wqkv.reshape(3, KD, 128, 128).transpose(2, 0, 1, 3)
        ).astype(BF)
        in_maps.append(
            {
                "xT": xT,
                "Wqkv": wqkv,
                "WpT": WpT,
                "bp": bp.reshape(1, D).astype(BF),
                "mask": mask,
                "idin": idin,
            }
        )
    return in_maps


# inputs identical across cores are passed replicated (shipped once, not 8x)
_REPLICATED = {"xT", "WpT", "bp", "mask", "idin"}

_runners = {}


def _get_runner(S):
    """Cached jitted SPMD callable for the built module."""
    if S in _runners:
        return _runners[S]
    import jax
    import concourse.mybir as _mybir
    from concourse import bass2jax
    from jax.experimental.shard_map import shard_map
    from jax.sharding import Mesh, PartitionSpec

    nc = get_nc(S)
    bass2jax.install_neuronx_cc_hook()

    in_names, out_names, out_avals = [], [], []
    partition_name = nc.partition_id_tensor.name if nc.partition_id_tensor else None
    for alloc in nc.m.functions[0].allocations:
        if not isinstance(alloc, _mybir.MemoryLocationSet):
            continue
        name = alloc.memorylocations[0].name
        if alloc.kind == "ExternalInput":
            if name != partition_name:
                in_names.append(name)
        elif alloc.kind == "ExternalOutput":
            out_names.append(name)
            out_avals.append(
                jax.core.ShapedArray(tuple(alloc.tensor_shape), _mybir.dt.np(alloc.dtype))
            )
    n_params = len(in_names)
    all_in_names = list(in_names) + list(out_names)
    if partition_name is not None:
        all_in_names.append(partition_name)

    def _body(*args):
        operands = list(args)
        if partition_name is not None:
            operands.append(bass2jax.partition_id_tensor())
        outs = bass2jax._bass_exec_p.bind(
            *operands,
            out_avals=tuple(out_avals),
            in_names=tuple(all_in_names),
            out_names=tuple(out_names),
            lowering_input_output_aliases=(),
            sim_require_finite=True,
            sim_require_nnan=True,
            nc=nc,
        )
        return tuple(outs)

    devices = jax.devices()[:NCORES]
    mesh = Mesh(np.asarray(devices), ("core",))
    in_specs = tuple(
        PartitionSpec() if nm in _REPLICATED else PartitionSpec("core")
        for nm in in_names
    ) + (PartitionSpec("core"),) * len(out_names)
    out_specs = (PartitionSpec("core"),) * len(out_names)
    donate = tuple(range(n_params, n_params + len(out_names)))
    fn = jax.jit(
        shard_map(_body, mesh=mesh, in_specs=in_specs, out_specs=out_specs, check_rep=False),
        donate_argnums=donate,
        keep_unused=True,
    )
    r = (fn, in_names, out_names, out_avals, mesh)
    _runners[S] = r
    return r


class _Res:
    def __init__(self, results):
        self.results = results
        self.exec_time_ns = None


def run(x, Wq, Wk, Wv, Wp, bp, timings=None):
    import time as _time

    S = x.shape[1]
    t0 = _time.perf_counter()
    fn, in_names, out_names, out_avals, mesh = _get_runner(S)
    t1 = _time.perf_counter()
    in_maps = prep_inputs(x, Wq, Wk, Wv, Wp, bp)
    t2 = _time.perf_counter()
    args = []
    for nm in in_names:
        if nm in _REPLICATED:
            args.append(in_maps[0][nm])
        else:
            args.append(np.concatenate([in_maps[c][nm] for c in range(NCORES)], axis=0))
    zero_outs = [
        np.zeros((NCORES * av.shape[0], *av.shape[1:]), av.dtype) for av in out_avals
    ]
    t3 = _time.perf_counter()
    out_arrs = fn(*args, *zero_outs)
    out_np = [np.asarray(o) for o in out_arrs]
    t4 = _time.perf_counter()
    results = [
        {
            nm: out_np[i].reshape(NCORES, *out_avals[i].shape)[c]
            for i, nm in enumerate(out_names)
        }
        for c in range(NCORES)
    ]
    if timings is not None:
        timings.update(
            runner=t1 - t0, prep=t2 - t1, concat=t3 - t2, exec=t4 - t3
        )
    return _assemble_y([results[c]["y"] for c in range(NCORES)], S), _Res(results)


def _assemble_y(per_core, S):
    """per-core y rows [b, p, j]: global q = SQ*(2p + j//64) + 64c + j%64."""
    SQ = 512
    NCH = S // SQ
    NP = NCH // 2
    out = np.empty((B, S, D), dtype=per_core[0].dtype)
    for c in range(NCORES):
        yc = per_core[c].reshape(B, NP, 128, D)
        for b in range(B):
            for p in range(NP):
                for half in range(2):
                    q0 = SQ * (2 * p + half) + 64 * c
                    out[b, q0 : q0 + 64, :] = yc[b, p, 64 * half : 64 * half + 64]
    return out


def kernel(x, Wq, Wk, Wv, Wp, bp):
    out, _ = run(x, Wq, Wk, Wv, Wp, bp)
    return out


# ---------------------------------------------------------------------------
# NTFF profiling support (test harness only; not needed for kernel()).
# ---------------------------------------------------------------------------

def _ntff_hook():
    import contextlib
    import ctypes

    lib = ctypes.CDLL("/opt/axon/libaxon_pjrt.so")
    lib.axon_start_nrt_profile.argtypes = [
        ctypes.POINTER(ctypes.c_int64),
        ctypes.c_size_t,
    ]
    lib.axon_start_nrt_profile.restype = ctypes.c_int64
    lib.axon_stop_nrt_profile.argtypes = [ctypes.c_char_p]
    lib.axon_stop_nrt_profile.restype = ctypes.c_int64

    @contextlib.contextmanager
    def _hook(output_dir, device_ids):
        import jax

        jax.devices()
        if device_ids:
            ids = (ctypes.c_int64 * len(device_ids))(*device_ids)
            rc = lib.axon_start_nrt_profile(ids, len(device_ids))
        else:
            rc = lib.axon_start_nrt_profile(None, 0)
        if rc != 0:
            raise RuntimeError(f"axon_start_nrt_profile rc={rc}")
        try:
            yield
        finally:
            n = lib.axon_stop_nrt_profile(str(output_dir).encode())
            print(f"profile: {n} file(s) written to {output_dir}")

    return _hook


def run_traced(x, Wq, Wk, Wv, Wp, bp, outdir=None, cores=(0,)):
    """Run once under NTFF profiling; returns (out, exec_time_ns, trace_path)."""
    import glob
    import tempfile

    import gauge.profiler
    from concourse._compat import FishPath

    S = x.shape[1]
    fn, in_names, out_names, out_avals, mesh = _get_runner(S)
    in_maps = prep_inputs(x, Wq, Wk, Wv, Wp, bp)
    args = []
    for nm in in_names:
        if nm in _REPLICATED:
            args.append(in_maps[0][nm])
        else:
            args.append(np.concatenate([in_maps[c][nm] for c in range(NCORES)], axis=0))
    zero_outs = [
        np.zeros((NCORES * av.shape[0], *av.shape[1:]), av.dtype) for av in out_avals
    ]
    # warm (compile + first exec)
    out_arrs = fn(*args, *zero_outs)
    _ = [np.asarray(o) for o in out_arrs]

    if outdir is None:
        outdir = tempfile.mkdtemp(prefix="ntff_")
    hook = _ntff_hook()
    zero_outs = [
        np.zeros((NCORES * av.shape[0], *av.shape[1:]), av.dtype) for av in out_avals
    ]
    with hook(outdir, list(cores)):
        out_arrs = fn(*args, *zero_outs)
        out_np = [np.asarray(o) for o in out_arrs]

    ntffs = glob.glob(f"{outdir}/*.ntff")
    if not ntffs:
        print(f"no NTFF files in {outdir}")
        return None, None, None
    nc = get_nc(S)
    profile = gauge.profiler.Profile(
        profile_path=FishPath(outdir),
        kernel_dev_mode=True,
        profile_on_exit=False,
        bass_kernel=nc.m,
        offline_processing=True,
        fname="*_body*",
        metadata={"artifacts_path": outdir},
    )
    results = profile.to_perfetto(model_index=tuple(range(len(cores))))
    exec_ns = max(r.exec_time_ns for r in results)
    yfull = _assemble_y(
        [out_np[out_names.index("y")].reshape(NCORES, -1, D)[c] for c in range(NCORES)],
        S,
    )
    return yfull, exec_ns, results[0].trace_path
